# revision 1
# baseline (speedup 1.0000x reference)
"""Lorenz Euler integration on Trainium2 (Bass/Tile).

Algorithm: the Euler recurrence
    x' = (1-dt*s)*x + dt*s*y
    y' = (1-dt)*y   + dt*x*(r-z)
    z' = (1-dt*b)*z + dt*x*y
is solved by global Gauss-Seidel sweeps over the whole trajectory. Each
component, given the others, is an exact constant-coefficient linear
recurrence solved in parallel with a blocked scan:
  phase1: per-chunk tensor_tensor_scan (state = a*state + f, init 0)
  phase2: chunk-boundary states E = T @ q via one PE matmul with a
          host-precomputed Toeplitz decay matrix (plus an s0 column)
  phase3: states = a^i * E + partial (scalar_tensor_tensor)
~30 sweeps reach the fp32 rounding floor (~7e-6 rel err vs the
sequential fp32 reference).

Layout: 3999 transitions in C=125 chunks x L=32 (one extra state 4000,
discarded). buf_k[125, 33]: col 0 = chunk-start state, cols 1..32 =
chunk states. All parameters (sigma, rho, beta, stats) are baked into
immediates / host-built constant tables at trace time.
"""
import sys
import numpy as np

sys.path.insert(0, "/opt/trn_rl_repo")

N = 4000
C = 125          # chunks (partitions)
L = 32           # steps per chunk
DT = 0.01
SWEEPS = 30
N_CORES = 8


def _build_consts(a_vals, s0_vals):
    """Host-precomputed fp32 tables: per component k the decay powers
    apow[c,i] = a^(i+1) and the chunk-propagation matrix lhsT with
    E = T @ qaug, T[c,j] = (a^L)^(c-1-j) for j<c, T[c,125] = a^(L*c)."""
    apows = np.zeros((3 * C, L), np.float32)
    tmats = np.zeros((3 * 128, 128), np.float32)
    for k, a in enumerate(a_vals):
        a = np.float64(a)
        apows[k * C:(k + 1) * C, :] = (a ** np.arange(1, L + 1))[None, :]
        aL = a ** L
        T = np.zeros((128, 128), np.float64)
        for c in range(C):
            j = np.arange(0, c)
            T[c, j] = aL ** (c - 1 - j)
            T[c, 125] = a ** (L * c)
        tmats[k * 128:(k + 1) * 128, :] = T.T.astype(np.float32)
    return apows, tmats


def _build_module(sigma, rho, beta, stats):
    import concourse.bass as bass
    import concourse.tile as tile
    import concourse.mybir as mybir
    from concourse import bacc

    FP32 = mybir.dt.float32
    mult = mybir.AluOpType.mult
    add = mybir.AluOpType.add
    sub = mybir.AluOpType.subtract

    a_vals = [1.0 - DT * sigma, 1.0 - DT, 1.0 - DT * beta]   # x, y, z
    # scaled state: xhat = -dt*x; y,z plain. Every forcing is one DVE op:
    #   f_y = (z-rho)*xhat = dt*x*(rho-z)            (y-scan: a*s + f)
    #   xhat*y = -dt*x*y -> z-scan uses op1=subtract (a*s - f)
    #   f_xhat = -dt^2*sigma*y                        (x-scan: a*s + f)
    s0 = [float(-DT * stats[0]), float(stats[1]), float(stats[2])]

    nc = bacc.Bacc("TRN2", target_bir_lowering=False)
    stats_col = nc.dram_tensor("stats_col", [3, 1], FP32, kind="ExternalInput")
    stats_row = nc.dram_tensor("stats_row", [1, 3], FP32, kind="ExternalInput")
    tmats_in = nc.dram_tensor("tmats", [3 * 128, 128], FP32, kind="ExternalInput")
    apows_in = nc.dram_tensor("apows", [3 * C, L], FP32, kind="ExternalInput")
    out_h = nc.dram_tensor("out", [N * 3], FP32, kind="ExternalOutput")

    with tile.TileContext(nc) as tc:
        with tc.tile_pool(name="sb", bufs=1) as pool, \
             tc.tile_pool(name="ps", bufs=1, space="PSUM") as psum:
            lhsT = [pool.tile([128, 128], FP32, tag=f"lhsT{k}", name=f"lhsT{k}") for k in range(3)]
            apow = [pool.tile([C, L], FP32, tag=f"apow{k}", name=f"apow{k}") for k in range(3)]
            acst = [pool.tile([C, L], FP32, tag=f"acst{k}", name=f"acst{k}") for k in range(3)]
            buf = [pool.tile([C, L + 1], FP32, tag=f"buf{k}", name=f"buf{k}") for k in range(3)]
            part = [pool.tile([128, L], FP32, tag=f"part{k}", name=f"part{k}") for k in range(3)]
            forc = [pool.tile([C, L], FP32, tag=f"forc{k}", name=f"forc{k}") for k in range(3)]
            e_ps = [psum.tile([128, 1], FP32, tag=f"eps{k}", name=f"eps{k}") for k in range(3)]
            st_row = pool.tile([1, 3], FP32, tag="strow")
            staging = pool.tile([C, 3 * L], FP32, tag="staging")

            # ---- init ----
            for k in range(3):
                nc.gpsimd.dma_start(lhsT[k][:], tmats_in[k * 128:(k + 1) * 128, :])
                nc.gpsimd.dma_start(apow[k][:], apows_in[k * C:(k + 1) * C, :])
                nc.vector.memset(acst[k][:], float(a_vals[k]))
                nc.vector.memset(buf[k][:], s0[k])
                nc.vector.memset(part[k][:], 0.0)
                # s0 feeds the matmul via qaug row 125
                nc.gpsimd.dma_start(part[k][125:126, L - 1:L], stats_col[k:k + 1, 0:1])
            nc.gpsimd.dma_start(st_row[:], stats_row[:])

            X, Y, Z = 0, 1, 2

            def solve(k, op1=add):
                """phase1 scan -> phase2 matmul -> E copy -> phase3."""
                nc.vector.tensor_tensor_scan(
                    part[k][0:C, :], acst[k][:], forc[k][:], 0.0, mult, op1)
                nc.tensor.matmul(e_ps[k][:], lhsT[k][:], part[k][:, L - 1:L],
                                 start=True, stop=True)
                nc.scalar.copy(buf[k][:, 0:1], e_ps[k][0:C, :])
                nc.vector.scalar_tensor_tensor(
                    buf[k][:, 1:L + 1], apow[k][:], e_ps[k][0:C, 0:1],
                    part[k][0:C, :], mult, add)

            for _ in range(SWEEPS):
                nc.vector.scalar_tensor_tensor(
                    forc[Y][:], buf[Z][:, 0:L], float(rho), buf[X][:, 0:L],
                    sub, mult)
                solve(Y)
                nc.vector.tensor_tensor(forc[Z][:], buf[X][:, 0:L],
                                        buf[Y][:, 0:L], mult)
                solve(Z, op1=sub)
                nc.vector.tensor_scalar_mul(forc[X][:], buf[Y][:, 0:L],
                                            float(-DT * DT * sigma))
                solve(X)

            # ---- output assembly: interleave x,y,z then DMA ----
            unscale = [-1.0 / DT, 1.0, 1.0]
            for k in range(3):
                nc.vector.tensor_scalar_mul(
                    staging[:].rearrange("c (i three) -> c i three", three=3)[:, :, k],
                    buf[k][:, 1:L + 1], unscale[k])
            nc.gpsimd.dma_start(out_h[0:3].rearrange("(a b) -> a b", a=1), st_row[:])
            nc.gpsimd.dma_start(
                out_h[3:3 + 124 * 96].rearrange("(c f) -> c f", f=96),
                staging[0:124, :])
            nc.gpsimd.dma_start(
                out_h[3 + 124 * 96:N * 3].rearrange("(a b) -> a b", a=1),
                staging[124:125, 0:93])

    nc.compile()
    return nc


def kernel(t, sigma, rho, beta, stats):
    from concourse.bass_utils import run_bass_kernel_spmd

    sigma = float(np.asarray(sigma).reshape(-1)[0])
    rho = float(np.asarray(rho).reshape(-1)[0])
    beta = float(np.asarray(beta).reshape(-1)[0])
    stats = np.asarray(stats, np.float32).reshape(3)

    a_vals = [1.0 - DT * sigma, 1.0 - DT, 1.0 - DT * beta]
    apows, tmats = _build_consts(a_vals, stats)
    nc = _build_module(sigma, rho, beta, stats)

    stats_scaled = np.array([-DT * stats[0], stats[1], stats[2]], np.float32)
    in_map = {
        "stats_col": stats_scaled.reshape(3, 1).copy(),
        "stats_row": stats.reshape(1, 3).copy(),
        "tmats": tmats,
        "apows": apows,
    }
    import os
    trace = bool(int(os.environ.get("LORENZ_TRACE", "0")))
    res = run_bass_kernel_spmd(nc, [dict(in_map) for _ in range(N_CORES)],
                               core_ids=list(range(N_CORES)), trace=trace)
    if trace and res.exec_time_ns is not None:
        print(f"HW exec time: {res.exec_time_ns} ns")
        if res.instructions_and_trace is not None:
            print("trace:", res.instructions_and_trace[1])
    out = res.results[0]["out"].reshape(N, 3).astype(np.float32)
    return out


if __name__ == "__main__":
    t = np.arange(0, 40, 0.01, dtype=np.float32)
    one = np.ones(1, np.float32)
    out = kernel(t=t, sigma=one, rho=one, beta=one, stats=np.ones(3, np.float32))
    print(out[:3], out[-2:])



# revision 3
# speedup vs baseline: 1.3929x; 1.3929x over previous
"""Lorenz Euler integration on Trainium2 (Bass/Tile).

Algorithm: Gauss-Seidel sweeps over the whole trajectory with exact
per-component linear-recurrence solves (blocked parallel scan + PE matmul
for the chunk-boundary chain). ~23 sweeps reach ~3e-3 rel err.

Scaled variables make every forcing one DVE op and eliminate the
x-forcing entirely:
    v = y                 v' = a_y v + (t - rho*r) * u
    t = r*z, r=-dt^2*s    t' = a_z t + (c_z*u) * v ,  c_z = r*dt^2*s
    u = x/(dt*s)          u' = a_x u + v            (forcing IS v)

Layout (C=64 chunks x L=63 steps = 4032 transitions, pad discarded):
  t_tile/u_tile [64,64] base-0; fzx [128,64]: rows 0-63 = v (X-scan
  forcing), rows 64-127 = z-forcing; part_zx rows 0-63 = x-partials,
  rows 64-127 = z-partials. Phase 3 is split (p3X base-0 ins, p3Z base-64
  ins) so all forcing products see same-base operands. s0 enters via a
  1.0 slot row (Y) / an accumulated [1,128] matmul (ZX).
"""
import sys
import numpy as np

sys.path.insert(0, "/opt/trn_rl_repo")

N = 4000
C = 64
L = 63
DT = 0.01
SWEEPS = 23
N_CORES = 8


def _host_consts(sigma, rho, beta, stats):
    """Packed [128, 512] fp32 table:
    cols 0:64    rows 0-63 apow_y;  cols 64:128 rows 0-63 apow_x,
                 rows 64-127 apow_z
    cols 128:256 lhsT_y ; cols 256:384 lhsT_zx ; cols 384:512 row 0 = e0_zx
    """
    a_y = 1.0 - DT
    a_z = 1.0 - DT * beta
    a_x = 1.0 - DT * sigma
    r = -DT * DT * sigma
    v0 = float(stats[1])
    t0 = float(r * stats[2])
    u0 = float(stats[0] / (DT * sigma))

    def pows(a):
        return (np.float64(a) ** np.arange(0, L + 1)).astype(np.float32)

    def tmat(a):
        """[C, C] lower-triangular decay: T[c, j] = aL^(c-1-j) for j < c."""
        aL = np.float64(a) ** L
        T = np.zeros((C, C), np.float64)
        for c in range(1, C):
            j = np.arange(0, c)
            T[c, j] = aL ** (c - 1 - j)
        return T

    consts = np.zeros((128, 512), np.float32)
    consts[0:C, 0:64] = pows(a_y)[None, :]
    consts[0:C, 64:128] = pows(a_x)[None, :]
    consts[C:128, 64:128] = pows(a_z)[None, :]

    # lhsT_y: E_y[c] = sum_{j<c} aLy^.. q_y[j] + aLy^c * v0 (rhs row 64 = 1)
    Ty = np.zeros((C, 128), np.float64)
    Ty[:, 0:C] = tmat(a_y)
    Ty[:, C] = (np.float64(a_y) ** L) ** np.arange(C) * v0
    consts[:, 128 + 0:128 + C] = Ty.T.astype(np.float32)[:, :]

    # lhsT_zx: out rows 0-63 = E_x (from in rows 0-63 = q_x),
    #          out rows 64-127 = E_z (from in rows 64-127 = q_z)
    Tzx = np.zeros((128, 128), np.float64)
    Tzx[0:C, 0:C] = tmat(a_x)
    Tzx[C:128, C:128] = tmat(a_z)
    consts[:, 256:384] = Tzx.T.astype(np.float32)

    # e0_zx row: s0 propagation for both blocks
    e0 = np.zeros(128, np.float64)
    e0[0:C] = (np.float64(a_x) ** L) ** np.arange(C) * u0
    e0[C:128] = (np.float64(a_z) ** L) ** np.arange(C) * t0
    consts[0, 384:512] = e0.astype(np.float32)
    return consts, (a_y, a_z, a_x, r, v0, t0, u0)


def _build_module(sigma, rho, beta, stats):
    import concourse.bass as bass
    import concourse.tile as tile
    import concourse.mybir as mybir
    from concourse import bacc

    FP32 = mybir.dt.float32
    mult = mybir.AluOpType.mult
    add = mybir.AluOpType.add
    sub = mybir.AluOpType.subtract

    _, (a_y, a_z, a_x, r, v0, t0, u0) = _host_consts(sigma, rho, beta, stats)
    rr = float(rho * r)
    c_z = float(r * DT * DT * sigma)

    nc = bacc.Bacc("TRN2", target_bir_lowering=False)
    consts_h = nc.dram_tensor("consts", [128, 512], FP32, kind="ExternalInput")
    out_h = nc.dram_tensor("out", [C * 189], FP32, kind="ExternalOutput")

    with tile.TileContext(nc) as tc:
        with tc.tile_pool(name="sb", bufs=1) as pool, \
             tc.tile_pool(name="ps", bufs=1, space="PSUM") as psum:
            csb = pool.tile([128, 512], FP32, tag="csb", name="csb")
            acst_y = pool.tile([C, L], FP32, tag="acsty", name="acsty")
            acst_zx = pool.tile([128, L], FP32, tag="acstzx", name="acstzx")
            part_y = pool.tile([128, L + 1], FP32, tag="party", name="party")
            part_zx = pool.tile([128, L + 1], FP32, tag="partzx", name="partzx")
            forc_y = pool.tile([C, L], FP32, tag="forcy", name="forcy")
            fzx = pool.tile([128, L + 1], FP32, tag="fzx", name="fzx")
            t_tile = pool.tile([C, L + 1], FP32, tag="tt", name="tt")
            u_tile = pool.tile([C, L + 1], FP32, tag="ut", name="ut")
            one_t = pool.tile([1, 1], FP32, tag="one", name="one")
            staging = pool.tile([C, 189], FP32, tag="staging", name="staging")
            e_y = psum.tile([128, 1], FP32, tag="ey", name="ey")
            e_zx = psum.tile([128, 1], FP32, tag="ezx", name="ezx")

            apow_y = csb[0:C, 0:64]
            apow_x = csb[0:C, 64:128]
            apow_z = csb[C:128, 64:128]
            lhsT_y = csb[:, 128:256]
            lhsT_zx = csb[:, 256:384]
            e0_zx = csb[0:1, 384:512]

            # ---- init ----
            nc.sync.dma_start(csb[:], consts_h[:, :])
            nc.vector.memset(acst_y[:], float(a_y))
            nc.vector.memset(acst_zx[0:C, :], float(a_x))
            nc.vector.memset(acst_zx[C:128, :], float(a_z))
            nc.vector.memset(part_y[:], 0.0)
            nc.vector.memset(part_y[C:C + 1, L:L + 1], 1.0)  # s0 slot
            nc.vector.memset(part_zx[:], 0.0)
            nc.vector.memset(t_tile[:], float(t0))
            nc.vector.memset(u_tile[:], float(u0))
            nc.vector.memset(fzx[0:C, :], float(v0))
            nc.vector.memset(fzx[C:128, :], 0.0)
            nc.vector.memset(one_t[:], 1.0)

            v_ap = fzx[0:C, 0:L]          # v states 0..62 (forcing for X)

            for _ in range(SWEEPS):
                # ---- round Y ----
                nc.vector.scalar_tensor_tensor(
                    forc_y[:], t_tile[:, 0:L], rr, u_tile[:, 0:L], sub, mult)
                nc.vector.tensor_tensor_scan(
                    part_y[0:C, 1:L + 1], acst_y[:], forc_y[:], 0.0, mult, add)
                nc.tensor.matmul(e_y[:], lhsT_y, part_y[:, L:L + 1],
                                 start=True, stop=True)
                nc.vector.scalar_tensor_tensor(
                    fzx[0:C, 0:L + 1], apow_y, e_y[0:C, 0:1],
                    part_y[0:C, 0:L + 1], mult, add)
                # ---- round ZX ----
                nc.vector.scalar_tensor_tensor(
                    fzx[C:128, 0:L], u_tile[:, 0:L], c_z, v_ap, mult, mult)
                nc.vector.tensor_tensor_scan(
                    part_zx[0:128, 1:L + 1], acst_zx[:], fzx[0:128, 0:L],
                    0.0, mult, add)
                nc.tensor.matmul(e_zx[:], lhsT_zx, part_zx[:, L:L + 1],
                                 start=True, stop=False)
                nc.tensor.matmul(e_zx[:], e0_zx, one_t[:],
                                 start=False, stop=True)
                nc.vector.scalar_tensor_tensor(
                    u_tile[:, 0:L + 1], apow_x, e_zx[0:C, 0:1],
                    part_zx[0:C, 0:L + 1], mult, add)
                nc.vector.scalar_tensor_tensor(
                    t_tile[:, 0:L + 1], apow_z, e_zx[C:128, 0:1],
                    part_zx[C:128, 0:L + 1], mult, add)

            # ---- output: interleave x,y,z ----
            sv = staging[:].rearrange("c (i three) -> c i three", three=3)
            nc.vector.tensor_scalar_mul(sv[:, :, 0], u_tile[:, 0:L],
                                        float(DT * sigma))
            nc.vector.tensor_scalar_mul(sv[:, :, 1], v_ap, 1.0)
            nc.vector.tensor_scalar_mul(sv[:, :, 2], t_tile[:, 0:L],
                                        float(1.0 / r))
            nc.sync.dma_start(
                out_h[:].rearrange("(c f) -> c f", f=189), staging[:])

    nc.compile()
    return nc


def kernel(t, sigma, rho, beta, stats):
    from concourse.bass_utils import run_bass_kernel_spmd

    sigma = float(np.asarray(sigma).reshape(-1)[0])
    rho = float(np.asarray(rho).reshape(-1)[0])
    beta = float(np.asarray(beta).reshape(-1)[0])
    stats = np.asarray(stats, np.float32).reshape(3)

    consts, _ = _host_consts(sigma, rho, beta, stats)
    nc = _build_module(sigma, rho, beta, stats)

    in_map = {"consts": consts}
    import os
    trace = bool(int(os.environ.get("LORENZ_TRACE", "0")))
    res = run_bass_kernel_spmd(nc, [dict(in_map) for _ in range(N_CORES)],
                               core_ids=list(range(N_CORES)), trace=trace)
    if trace and res.exec_time_ns is not None:
        print(f"HW exec time: {res.exec_time_ns} ns")
    out = res.results[0]["out"][:N * 3].reshape(N, 3).astype(np.float32)
    return out


if __name__ == "__main__":
    t = np.arange(0, 40, 0.01, dtype=np.float32)
    one = np.ones(1, np.float32)
    out = kernel(t=t, sigma=one, rho=one, beta=one, stats=np.ones(3, np.float32))
    print(out[:3], out[-2:])


# revision 5
# speedup vs baseline: 1.5132x; 1.0863x over previous
"""Lorenz Euler integration on Trainium2 (Bass/Tile).

Algorithm: Gauss-Seidel sweeps over the whole trajectory with exact
per-component linear-recurrence solves (blocked parallel scan + PE matmul
for the chunk-boundary chain). 23 sweeps reach ~3e-3 rel err.

Scaled variables make every forcing one DVE op and eliminate the
x-forcing entirely:
    v = y                 v' = a_y v + (t - rho*r) * u
    t = r*z, r=-dt^2*s    t' = a_z t + (c_z*u) * v ,  c_z = r*dt^2*s
    u = x/(dt*s)          u' = a_x u + v            (forcing IS v)

Layout (C=64 chunks x L=63 steps = 4032 transitions, pad discarded):
  t_tile/u_tile [64,64] base-0; fzx [128,64]: rows 0-63 = v (X-scan
  forcing), rows 64-127 = z-forcing; part_zx rows 0-63 = x-partials,
  rows 64-127 = z-partials. Phase 3 is split (p3X base-0 ins, p3Z base-64
  ins) so all forcing products see same-base operands. s0 enters via a
  1.0 slot row (Y) / an accumulated [1,128] matmul (ZX).

Scheduling: the z-update (p3Z) of sweep k-1 is deferred into sweep k's
mm_y wait window; the y-forcing then uses a one-sweep-older t, which
converges in the same 23 sweeps (z couples weakly into y). Sweep 0's
y-forcing is a host constant, so t/v tiles need no init.
"""
import sys
import numpy as np

sys.path.insert(0, "/opt/trn_rl_repo")

N = 4000
C = 64
L = 63
DT = 0.01
SWEEPS = 23
N_CORES = 8

# csb column map
A_Y0 = 0          # apow_y   [64 cols]  rows 0-63
A_X0 = 64         # apow_x   [64 cols]  rows 0-63
A_Z0 = 64         # apow_z   same cols, rows 64-127
LT_Y0 = 128       # lhsT_y   [128 cols]
LT_ZX0 = 256      # lhsT_zx  [128 cols]
E0_ZX0 = 384      # e0_zx    [128 cols] row 0
AC_Y0 = 512       # acst_y   [63 cols]  rows 0-63 (= a)
AC_ZX0 = 575      # acst_zx  [63 cols]  rows 0-63 = a_x, rows 64-127 = a_z
ONE0 = 638        # 1.0 at row 0
F00 = 639         # forcY sweep-0 host constant [63 cols] rows 0-63
NCOLS = 702


def _host_consts(sigma, rho, beta, stats):
    a_y = 1.0 - DT
    a_z = 1.0 - DT * beta
    a_x = 1.0 - DT * sigma
    r = -DT * DT * sigma
    v0 = float(stats[1])
    t0 = float(r * stats[2])
    u0 = float(stats[0] / (DT * sigma))
    rr = rho * r

    def pows(a):
        return (np.float64(a) ** np.arange(0, L + 1)).astype(np.float32)

    def tmat(a):
        aL = np.float64(a) ** L
        T = np.zeros((C, C), np.float64)
        for c in range(1, C):
            j = np.arange(0, c)
            T[c, j] = aL ** (c - 1 - j)
        return T

    consts = np.zeros((128, NCOLS), np.float32)
    consts[0:C, A_Y0:A_Y0 + 64] = pows(a_y)[None, :]
    consts[0:C, A_X0:A_X0 + 64] = pows(a_x)[None, :]
    consts[C:128, A_Z0:A_Z0 + 64] = pows(a_z)[None, :]

    Ty = np.zeros((C, 128), np.float64)
    Ty[:, 0:C] = tmat(a_y)
    Ty[:, C] = (np.float64(a_y) ** L) ** np.arange(C) * v0
    consts[:, LT_Y0:LT_Y0 + C] = Ty.T.astype(np.float32)

    Tzx = np.zeros((128, 128), np.float64)
    Tzx[0:C, 0:C] = tmat(a_x)
    Tzx[C:128, C:128] = tmat(a_z)
    consts[:, LT_ZX0:LT_ZX0 + 128] = Tzx.T.astype(np.float32)

    e0 = np.zeros(128, np.float64)
    e0[0:C] = (np.float64(a_x) ** L) ** np.arange(C) * u0
    e0[C:128] = (np.float64(a_z) ** L) ** np.arange(C) * t0
    consts[0, E0_ZX0:E0_ZX0 + 128] = e0.astype(np.float32)

    consts[0:C, AC_Y0:AC_Y0 + L] = np.float32(a_y)
    consts[0:C, AC_ZX0:AC_ZX0 + L] = np.float32(a_x)
    consts[C:128, AC_ZX0:AC_ZX0 + L] = np.float32(a_z)
    consts[0, ONE0] = 1.0
    consts[0:C, F00:F00 + L] = np.float32((t0 - rr) * u0)
    return consts, (a_y, a_z, a_x, r, v0, t0, u0)


def _build_module(sigma, rho, beta, stats):
    import concourse.bass as bass
    import concourse.tile as tile
    import concourse.mybir as mybir
    from concourse import bacc

    FP32 = mybir.dt.float32
    mult = mybir.AluOpType.mult
    add = mybir.AluOpType.add
    sub = mybir.AluOpType.subtract

    _, (a_y, a_z, a_x, r, v0, t0, u0) = _host_consts(sigma, rho, beta, stats)
    rr = float(rho * r)
    c_z = float(r * DT * DT * sigma)

    nc = bacc.Bacc("TRN2", target_bir_lowering=False)
    consts_h = nc.dram_tensor("consts", [128, NCOLS], FP32, kind="ExternalInput")
    out_h = nc.dram_tensor("out", [C * 189], FP32, kind="ExternalOutput")

    with tile.TileContext(nc) as tc:
        with tc.tile_pool(name="sb", bufs=1) as pool, \
             tc.tile_pool(name="ps", bufs=1, space="PSUM") as psum:
            csb = pool.tile([128, NCOLS], FP32, tag="csb", name="csb")
            part_y = pool.tile([128, L + 1], FP32, tag="party", name="party")
            part_zx = pool.tile([128, L + 1], FP32, tag="partzx", name="partzx")
            forc_y = pool.tile([C, L], FP32, tag="forcy", name="forcy")
            fzx = pool.tile([128, L + 1], FP32, tag="fzx", name="fzx")
            t_tile = pool.tile([C, L + 1], FP32, tag="tt", name="tt")
            u_tile = pool.tile([C, L + 1], FP32, tag="ut", name="ut")
            staging = pool.tile([C, 189], FP32, tag="staging", name="staging")
            e_y = psum.tile([128, 1], FP32, tag="ey", name="ey")
            e_zx = psum.tile([128, 1], FP32, tag="ezx", name="ezx")

            apow_y = csb[0:C, A_Y0:A_Y0 + 64]
            apow_x = csb[0:C, A_X0:A_X0 + 64]
            apow_z = csb[C:128, A_Z0:A_Z0 + 64]
            lhsT_y = csb[:, LT_Y0:LT_Y0 + 128]
            lhsT_zx = csb[:, LT_ZX0:LT_ZX0 + 128]
            e0_zx = csb[0:1, E0_ZX0:E0_ZX0 + 128]
            acst_y = csb[0:C, AC_Y0:AC_Y0 + L]
            acst_zx = csb[0:128, AC_ZX0:AC_ZX0 + L]
            one_t = csb[0:1, ONE0:ONE0 + 1]
            forc0 = csb[0:C, F00:F00 + L]

            # ---- init ----
            nc.sync.dma_start(csb[:], consts_h[:, :])
            nc.vector.memset(part_y[:], 0.0)
            nc.vector.memset(part_y[C:C + 1, L:L + 1], 1.0)  # s0 slot
            nc.vector.memset(part_zx[:], 0.0)
            nc.vector.memset(u_tile[:], float(u0))

            v_ap = fzx[0:C, 0:L]

            def round_y(forc):
                nc.vector.tensor_tensor_scan(
                    part_y[0:C, 1:L + 1], acst_y, forc, 0.0, mult, add)
                nc.tensor.matmul(e_y[:], lhsT_y, part_y[:, L:L + 1],
                                 start=True, stop=True)

            def p3y_and_zx():
                nc.vector.scalar_tensor_tensor(
                    fzx[0:C, 0:L + 1], apow_y, e_y[0:C, 0:1],
                    part_y[0:C, 0:L + 1], mult, add)
                nc.vector.scalar_tensor_tensor(
                    fzx[C:128, 0:L], u_tile[:, 0:L], c_z, v_ap, mult, mult)
                nc.vector.tensor_tensor_scan(
                    part_zx[0:128, 1:L + 1], acst_zx, fzx[0:128, 0:L],
                    0.0, mult, add)
                nc.tensor.matmul(e_zx[:], lhsT_zx, part_zx[:, L:L + 1],
                                 start=True, stop=False)
                nc.tensor.matmul(e_zx[:], e0_zx, one_t, start=False, stop=True)
                nc.vector.scalar_tensor_tensor(
                    u_tile[:, 0:L + 1], apow_x, e_zx[0:C, 0:1],
                    part_zx[0:C, 0:L + 1], mult, add)

            def p3z():
                nc.vector.scalar_tensor_tensor(
                    t_tile[:, 0:L + 1], apow_z, e_zx[C:128, 0:1],
                    part_zx[C:128, 0:L + 1], mult, add)

            # sweep 0: y-forcing is a host constant; p3Z runs in place
            round_y(forc0)
            p3y_and_zx()
            p3z()

            for k in range(1, SWEEPS):
                nc.vector.scalar_tensor_tensor(
                    forc_y[:], t_tile[:, 0:L], rr, u_tile[:, 0:L], sub, mult)
                round_y(forc_y[:])
                if k >= 2:
                    p3z()  # deferred z-update of sweep k-1 (fills mm_y wait)
                p3y_and_zx()
            p3z()

            # ---- output: interleave x,y,z ----
            sv = staging[:].rearrange("c (i three) -> c i three", three=3)
            nc.vector.tensor_scalar_mul(sv[:, :, 0], u_tile[:, 0:L],
                                        float(DT * sigma))
            nc.vector.tensor_scalar_mul(sv[:, :, 1], v_ap, 1.0)
            nc.vector.tensor_scalar_mul(sv[:, :, 2], t_tile[:, 0:L],
                                        float(1.0 / r))
            nc.sync.dma_start(
                out_h[:].rearrange("(c f) -> c f", f=189), staging[:])

    nc.compile()
    return nc


def kernel(t, sigma, rho, beta, stats):
    from concourse.bass_utils import run_bass_kernel_spmd

    sigma = float(np.asarray(sigma).reshape(-1)[0])
    rho = float(np.asarray(rho).reshape(-1)[0])
    beta = float(np.asarray(beta).reshape(-1)[0])
    stats = np.asarray(stats, np.float32).reshape(3)

    consts, _ = _host_consts(sigma, rho, beta, stats)
    nc = _build_module(sigma, rho, beta, stats)

    in_map = {"consts": consts}
    import os
    trace = bool(int(os.environ.get("LORENZ_TRACE", "0")))
    res = run_bass_kernel_spmd(nc, [dict(in_map) for _ in range(N_CORES)],
                               core_ids=list(range(N_CORES)), trace=trace)
    if trace and res.exec_time_ns is not None:
        print(f"HW exec time: {res.exec_time_ns} ns")
    out = res.results[0]["out"][:N * 3].reshape(N, 3).astype(np.float32)
    return out


if __name__ == "__main__":
    t = np.arange(0, 40, 0.01, dtype=np.float32)
    one = np.ones(1, np.float32)
    out = kernel(t=t, sigma=one, rho=one, beta=one, stats=np.ones(3, np.float32))
    print(out[:3], out[-2:])


# revision 6
# speedup vs baseline: 1.6533x; 1.0926x over previous
"""Lorenz Euler integration on Trainium2 (Bass/Tile).

Algorithm: Gauss-Seidel sweeps over the whole trajectory with exact
per-component linear-recurrence solves (blocked parallel scan + PE matmul
for the chunk-boundary chain). 23 sweeps reach ~3e-3 rel err.

Scaled variables make every forcing one DVE op and eliminate the
x-forcing entirely:
    v = y                 v' = a_y v + (t - rho*r) * u
    t = r*z, r=-dt^2*s    t' = a_z t + (c_z*u) * v ,  c_z = r*dt^2*s
    u = x/(dt*s)          u' = a_x u + v            (forcing IS v)

Layout (C=64 chunks x L=63 steps = 4032 transitions, pad discarded):
  t_tile/u_tile [64,64] base-0; fzx [128,64]: rows 0-63 = v (X-scan
  forcing), rows 64-127 = z-forcing; part_zx rows 0-63 = x-partials,
  rows 64-127 = z-partials. Phase 3 is split (p3X base-0 ins, p3Z base-64
  ins) so all forcing products see same-base operands. s0 enters via a
  1.0 slot row (Y) / an accumulated [1,128] matmul (ZX).

Scheduling: the z-update (p3Z) of sweep k-1 is deferred into sweep k's
mm_y wait window; the y-forcing then uses a one-sweep-older t, which
converges in the same 23 sweeps (z couples weakly into y). Sweep 0's
y-forcing is a host constant, so t/v tiles need no init.
"""
import sys
import numpy as np

sys.path.insert(0, "/opt/trn_rl_repo")

N = 4000
C = 64
L = 63
DT = 0.01
SWEEPS = 21
N_CORES = 8

# csb column map (a-coefficient tiles / sweep-0 forcing are memsets instead,
# so the first scans run while this table is still in flight)
A_Y0 = 0          # apow_y   [64 cols]  rows 0-63
A_X0 = 64         # apow_x   [64 cols]  rows 0-63
A_Z0 = 64         # apow_z   same cols, rows 64-127
LT_Y0 = 128       # lhsT_y   [64 cols]
LT_ZX0 = 192      # lhsT_zx  [128 cols]
E0_ZX0 = 320      # e0_zx    [128 cols] row 0
NCOLS = 448


def _host_consts(sigma, rho, beta, stats):
    a_y = 1.0 - DT
    a_z = 1.0 - DT * beta
    a_x = 1.0 - DT * sigma
    r = -DT * DT * sigma
    v0 = float(stats[1])
    t0 = float(r * stats[2])
    u0 = float(stats[0] / (DT * sigma))
    rr = rho * r

    def pows(a):
        return (np.float64(a) ** np.arange(0, L + 1)).astype(np.float32)

    def tmat(a):
        aL = np.float64(a) ** L
        T = np.zeros((C, C), np.float64)
        for c in range(1, C):
            j = np.arange(0, c)
            T[c, j] = aL ** (c - 1 - j)
        return T

    consts = np.zeros((128, NCOLS), np.float32)
    consts[0:C, A_Y0:A_Y0 + 64] = pows(a_y)[None, :]
    consts[0:C, A_X0:A_X0 + 64] = pows(a_x)[None, :]
    consts[C:128, A_Z0:A_Z0 + 64] = pows(a_z)[None, :]

    Ty = np.zeros((C, 128), np.float64)
    Ty[:, 0:C] = tmat(a_y)
    Ty[:, C] = (np.float64(a_y) ** L) ** np.arange(C) * v0
    consts[:, LT_Y0:LT_Y0 + 64] = Ty.T.astype(np.float32)[:, 0:64]

    Tzx = np.zeros((128, 128), np.float64)
    Tzx[0:C, 0:C] = tmat(a_x)
    Tzx[C:128, C:128] = tmat(a_z)
    consts[:, LT_ZX0:LT_ZX0 + 128] = Tzx.T.astype(np.float32)

    e0 = np.zeros(128, np.float64)
    e0[0:C] = (np.float64(a_x) ** L) ** np.arange(C) * u0
    e0[C:128] = (np.float64(a_z) ** L) ** np.arange(C) * t0
    consts[0, E0_ZX0:E0_ZX0 + 128] = e0.astype(np.float32)

    return consts, (a_y, a_z, a_x, r, v0, t0, u0)


def _build_module(sigma, rho, beta, stats):
    import concourse.bass as bass
    import concourse.tile as tile
    import concourse.mybir as mybir
    from concourse import bacc

    FP32 = mybir.dt.float32
    mult = mybir.AluOpType.mult
    add = mybir.AluOpType.add
    sub = mybir.AluOpType.subtract

    _, (a_y, a_z, a_x, r, v0, t0, u0) = _host_consts(sigma, rho, beta, stats)
    rr = float(rho * r)
    c_z = float(r * DT * DT * sigma)

    nc = bacc.Bacc("TRN2", target_bir_lowering=False)
    consts_h = nc.dram_tensor("consts", [128, NCOLS], FP32, kind="ExternalInput")
    out_h = nc.dram_tensor("out", [C * 189], FP32, kind="ExternalOutput")

    with tile.TileContext(nc) as tc:
        with tc.tile_pool(name="sb", bufs=1) as pool, \
             tc.tile_pool(name="ps", bufs=1, space="PSUM") as psum:
            csb = pool.tile([128, NCOLS], FP32, tag="csb", name="csb")
            part_y = pool.tile([128, L + 1], FP32, tag="party", name="party")
            part_zx = pool.tile([128, L + 1], FP32, tag="partzx", name="partzx")
            forc_y = pool.tile([C, L], FP32, tag="forcy", name="forcy")
            fzx = pool.tile([128, L + 1], FP32, tag="fzx", name="fzx")
            t_tile = pool.tile([C, L + 1], FP32, tag="tt", name="tt")
            u_tile = pool.tile([C, L + 1], FP32, tag="ut", name="ut")
            staging = pool.tile([C, 189], FP32, tag="staging", name="staging")
            acst_y = pool.tile([C, L], FP32, tag="acsty", name="acsty")
            acst_zx = pool.tile([128, L], FP32, tag="acstzx", name="acstzx")
            one_t = pool.tile([1, 1], FP32, tag="one", name="one")
            forc0 = pool.tile([C, L], FP32, tag="forc0", name="forc0")
            e_y = psum.tile([128, 1], FP32, tag="ey", name="ey")
            e_zx = psum.tile([128, 1], FP32, tag="ezx", name="ezx")

            apow_y = csb[0:C, A_Y0:A_Y0 + 64]
            apow_x = csb[0:C, A_X0:A_X0 + 64]
            apow_z = csb[C:128, A_Z0:A_Z0 + 64]
            lhsT_y = csb[:, LT_Y0:LT_Y0 + 64]
            lhsT_zx = csb[:, LT_ZX0:LT_ZX0 + 128]
            e0_zx = csb[0:1, E0_ZX0:E0_ZX0 + 128]

            # ---- init (memsets overlap with the consts DMA) ----
            nc.sync.dma_start(csb[:], consts_h[:, :])
            nc.vector.memset(part_y[:], 0.0)
            nc.vector.memset(part_y[C:C + 1, L:L + 1], 1.0)  # s0 slot
            nc.vector.memset(part_zx[:], 0.0)
            nc.vector.memset(u_tile[:], float(u0))
            nc.vector.memset(acst_y[:], float(a_y))
            nc.vector.memset(acst_zx[0:C, :], float(a_x))
            nc.vector.memset(acst_zx[C:128, :], float(a_z))
            nc.vector.memset(one_t[:], 1.0)
            nc.vector.memset(forc0[:], float((t0 - rr) * u0))

            v_ap = fzx[0:C, 0:L]

            def round_y(forc):
                nc.vector.tensor_tensor_scan(
                    part_y[0:C, 1:L + 1], acst_y[:], forc, 0.0, mult, add)
                nc.tensor.matmul(e_y[0:C, :], lhsT_y, part_y[:, L:L + 1],
                                 start=True, stop=True)

            def p3y_and_zx():
                nc.vector.scalar_tensor_tensor(
                    fzx[0:C, 0:L + 1], apow_y, e_y[0:C, 0:1],
                    part_y[0:C, 0:L + 1], mult, add)
                nc.vector.scalar_tensor_tensor(
                    fzx[C:128, 0:L], u_tile[:, 0:L], c_z, v_ap, mult, mult)
                nc.vector.tensor_tensor_scan(
                    part_zx[0:128, 1:L + 1], acst_zx[:], fzx[0:128, 0:L],
                    0.0, mult, add)
                nc.tensor.matmul(e_zx[:], lhsT_zx, part_zx[:, L:L + 1],
                                 start=True, stop=False)
                nc.tensor.matmul(e_zx[:], e0_zx, one_t[:], start=False,
                                 stop=True)
                nc.vector.scalar_tensor_tensor(
                    u_tile[:, 0:L + 1], apow_x, e_zx[0:C, 0:1],
                    part_zx[0:C, 0:L + 1], mult, add)

            def p3z():
                nc.vector.scalar_tensor_tensor(
                    t_tile[:, 0:L + 1], apow_z, e_zx[C:128, 0:1],
                    part_zx[C:128, 0:L + 1], mult, add)

            # sweep 0: y-forcing is a host constant; p3Z runs in place
            round_y(forc0[:])
            p3y_and_zx()
            p3z()

            for k in range(1, SWEEPS):
                nc.vector.scalar_tensor_tensor(
                    forc_y[:], t_tile[:, 0:L], rr, u_tile[:, 0:L], sub, mult)
                round_y(forc_y[:])
                if k >= 2:
                    p3z()  # deferred z-update of sweep k-1 (fills mm_y wait)
                p3y_and_zx()
            p3z()

            # ---- output: interleave x,y,z ----
            sv = staging[:].rearrange("c (i three) -> c i three", three=3)
            nc.gpsimd.tensor_scalar_mul(sv[:, :, 1], v_ap, 1.0)
            nc.gpsimd.tensor_scalar_mul(sv[:, :, 0], u_tile[:, 0:L],
                                        float(DT * sigma))
            nc.gpsimd.tensor_scalar_mul(sv[:, :, 2], t_tile[:, 0:L],
                                        float(1.0 / r))
            nc.sync.dma_start(
                out_h[:].rearrange("(c f) -> c f", f=189), staging[:])

    nc.compile()
    return nc


def kernel(t, sigma, rho, beta, stats):
    from concourse.bass_utils import run_bass_kernel_spmd

    sigma = float(np.asarray(sigma).reshape(-1)[0])
    rho = float(np.asarray(rho).reshape(-1)[0])
    beta = float(np.asarray(beta).reshape(-1)[0])
    stats = np.asarray(stats, np.float32).reshape(3)

    consts, _ = _host_consts(sigma, rho, beta, stats)
    nc = _build_module(sigma, rho, beta, stats)

    in_map = {"consts": consts}
    import os
    trace = bool(int(os.environ.get("LORENZ_TRACE", "0")))
    res = run_bass_kernel_spmd(nc, [dict(in_map) for _ in range(N_CORES)],
                               core_ids=list(range(N_CORES)), trace=trace)
    if trace and res.exec_time_ns is not None:
        print(f"HW exec time: {res.exec_time_ns} ns")
    out = res.results[0]["out"][:N * 3].reshape(N, 3).astype(np.float32)
    return out


if __name__ == "__main__":
    t = np.arange(0, 40, 0.01, dtype=np.float32)
    one = np.ones(1, np.float32)
    out = kernel(t=t, sigma=one, rho=one, beta=one, stats=np.ones(3, np.float32))
    print(out[:3], out[-2:])
